# revision 33
# baseline (speedup 1.0000x reference)
"""Distributed Trainium2 (Bass/Tile) kernel for the GNN message-passing problem.

Strategy (8 NeuronCores, SPMD):
  * Nodes are partitioned across the 8 cores (12500 each). Within a core,
    local nodes are sorted by total in-degree desc, then each 1024-node
    band is re-sorted by the cross-scale degree difference, so every
    128-node tile is degree-homogeneous for BOTH edge scales -> the
    round-based gather below pads few slots (the per-tile round count is
    the tile's max per-scale in-degree).
  * Small weight tensors are replicated to every core.
  * Per aggregation block: each core computes fc_1 features for its local
    nodes, the shards are exchanged with an AllGather into a replicated
    [N_tbl, 128] bf16 DRAM table, and the scatter_max is computed locally:
    round r gathers the r-th incoming edge of every local node with one
    [128,1]-offset indirect DMA (pad slots point at zeroed table rows),
    and a halving tree of tensor_max ops reduces the rounds into the agg
    tile.  relu(...) >= 0 makes the zero rows the identity of the max.
  * The gather DMAs are the serial resource (one SWDGE descriptor-gen per
    round, ~1.1 us each on the Pool engine).  To keep Pool saturated, the
    per-tile work of three pipeline stages is INTERLEAVED in emission
    order: gather(k,t) ; fc2(k,t) ; fc1(k+1,t) — so the PE/DVE/ACT work
    of block k's tail and block k+1's fc1 runs in the shadow of block k's
    remaining gather rounds, and the next AllGather fires as soon as the
    last fc1 tile is written.

Host-side prep only touches index tensors / layout (graph partitioning),
never the float data.
"""

import sys

for _p in ("/opt/trn_rl_repo", "/root/.axon_site/_ro/trn_rl_repo"):
    if _p not in sys.path:
        sys.path.append(_p)

import numpy as np

import concourse.bass as bass
import concourse.tile as tile
from concourse import mybir
from concourse.masks import make_identity
from concourse.tile import ScopedClock


class _TileContext(tile.TileContext):
    """TileContext whose tail drain carries at most one sync wait.

    The walrus build in this container rejects TPB_CTRL instructions with
    more than a couple of sync waits ("Too many sync wait commands"), and
    the stock tail drain waits on every live semaphore at once.  Split the
    waits onto single-wait NOPs in front of the drain instead.
    """

    def _drain_and_barrier(self, tick_clock, wait_clock):
        nc = self.nc
        probe = nc.sync.nop(nofuse=True)
        wait_clock.add_sem_waits(probe.ins,
                                 ScopedClock({None: tick_clock.global_clock}))
        si = probe.ins.sync_info
        waits = list(si.on_wait or []) if si else []
        upd = list(si.on_update or []) if si else []
        probe.ins.sync_info = mybir.SyncInfo(on_wait=waits[:1], on_update=upd)
        for w in waits[1:]:
            n = nc.sync.nop(nofuse=True)
            n.ins.sync_info = mybir.SyncInfo(on_wait=[w], on_update=[])
        nc.sync.drain()
        nc.all_engine_barrier()
        assert self.sems is not None
        popped = nc._tile_sem_poison_stack.pop()
        assert popped is self._sem_poison
        nc.clear_and_free_semaphores(list(self.sems.allocated().values()))
        nc.all_engine_barrier()

P = 128           # partitions / tile rows
C = 128           # channels (N_MAP)
NCORES = 8
EPS = 1e-5

AFT = mybir.ActivationFunctionType
ALU = mybir.AluOpType

FULL_GEOM = dict(n_nodes=100000, n_feat=22, n_scales=2, n_blk=2)

TBL_BF16 = True  # exchange/gather the fc1 table in bf16


def _host_prep(u, v, n_nodes):
    """Compute per-core node ordering and per-scale gather index arrays.

    Returns dict with:
      order : [NCORES, NPL] global node id per local slot (or -1 for pad)
      calls : per-scale list of per-tile (col, K)
      cols16: per-scale total index columns
      idx   : per-scale list of per-core [P, cols] int32 gather indices
      TL, NPL, Kslab (max per-tile round count), HALF
    """
    n_scales = u.shape[0]
    nloc = n_nodes // NCORES
    TL = (nloc + P - 1) // P
    NPL = TL * P
    SH = NPL + P          # shard rows in the table (incl. P zero rows)
    HALF = (TL // 2) * P  # first-half slots exchanged by the early AllGather

    u = [np.asarray(u[i]).astype(np.int64) for i in range(n_scales)]
    v = [np.asarray(v[i]).astype(np.int64) for i in range(n_scales)]

    deg = np.zeros((n_scales, n_nodes), np.int64)
    for i in range(n_scales):
        deg[i] = np.bincount(v[i], minlength=n_nodes)
    score = deg.max(axis=0)
    score2 = deg[0] - deg[1] if n_scales > 1 else deg[0]

    order = np.full((NCORES, NPL), -1, np.int64)
    slot = np.zeros(n_nodes, np.int64)
    for c in range(NCORES):
        ids = np.arange(c * nloc, min((c + 1) * nloc, n_nodes))
        o = ids[np.argsort(-score[ids], kind="stable")]
        parts = []
        for b in range(0, len(o), 1024):
            blk = o[b:b + 1024]
            parts.append(blk[np.argsort(-score2[blk], kind="stable")])
        o = np.concatenate(parts) if parts else o
        order[c, : len(o)] = o
        slot[o] = np.arange(len(o))

    core_of = np.arange(n_nodes) // nloc
    np.minimum(core_of, NCORES - 1, out=core_of)

    calls_all, cols_all, idx_all = [], [], []
    for i in range(n_scales):
        ui, vi = u[i], v[i]
        dst_core = core_of[vi]
        dst_slot = slot[vi]
        s_u = slot[ui]
        c_u = core_of[ui]
        src_tbl = np.where(
            s_u < HALF, c_u * HALF + s_u,
            NCORES * HALF + c_u * (SH - HALF) + (s_u - HALF)).astype(np.int64)

        per_core = []
        K = np.zeros((NCORES, TL), np.int64)
        for c in range(NCORES):
            sel = np.nonzero(dst_core == c)[0]
            ls = dst_slot[sel]
            srt = np.argsort(ls, kind="stable")
            ls_s = ls[srt]
            first = np.searchsorted(ls_s, ls_s, side="left")
            rank = np.arange(len(ls_s)) - first
            per_core.append((sel[srt], ls_s, rank))
            cnt = np.bincount(ls, minlength=NPL)
            K[c] = cnt.reshape(TL, P).max(axis=1)

        Kt = K.max(axis=0)                       # [TL] shared program shape
        calls = []
        col = 0
        col_of_tile = np.zeros(TL, np.int64)
        for t in range(TL):
            kk = int(Kt[t])
            calls.append((col, kk))
            col_of_tile[t] = col
            col += kk
        cols = col

        per_core_idx = []
        for c in range(NCORES):
            eidx, ls_s, rank = per_core[c]
            pad0 = NCORES * HALF + (NPL - HALF)
            arr = np.broadcast_to(
                (pad0 + np.arange(P, dtype=np.int32))[:, None],
                (P, max(cols, 1))).copy()
            t_e = ls_s // P
            p_e = ls_s % P
            arr[p_e, col_of_tile[t_e] + rank] = src_tbl[eidx].astype(np.int32)
            per_core_idx.append(arr)

        calls_all.append(calls)
        cols_all.append(cols)
        idx_all.append(per_core_idx)

    Kslab = max((kk for cl in calls_all for (_, kk) in cl), default=1)
    return dict(order=order, calls=calls_all, cols16=cols_all, idx=idx_all,
                TL=TL, NPL=NPL, nloc=nloc, Kslab=Kslab, HALF=HALF)


def _legalize_waits(nc, maxw=1):
    """Split multi-wait instructions into single-wait NOPs + the instruction."""
    f = nc.m.functions[0]
    n_split = 0
    for blk in f.blocks:
        insts = list(blk.instructions)
        if not any(i.sync_info and i.sync_info.on_wait
                   and len(i.sync_info.on_wait) > maxw for i in insts):
            continue
        new = []
        for inst in insts:
            si = inst.sync_info
            waits = list(si.on_wait) if si and si.on_wait else []
            if len(waits) > maxw:
                for j, w in enumerate(waits[:-maxw]):
                    nop = mybir.InstNoOp(
                        name=f"{inst.name}-sw{j}", engine=inst.engine,
                        ins=[], outs=[],
                        sync_info=mybir.SyncInfo(on_wait=[w], on_update=[]))
                    nc.register_instruction(nop, overwrite=True)
                    new.append(nop)
                    n_split += 1
                inst.sync_info = mybir.SyncInfo(
                    on_wait=waits[-maxw:], on_update=list(si.on_update or []))
            new.append(inst)
        blk.instructions = new
    return n_split


def _bc(x):
    """broadcast a [C] vector to a [P, 4*C] f32 tile (4x tiled for the
    4-tile batched GroupNorm)."""
    return np.ascontiguousarray(np.tile(np.broadcast_to(
        np.asarray(x, np.float32).reshape(1, C), (P, C)), (1, 4)))


# ---------------------------------------------------------------------------
# program builder
# ---------------------------------------------------------------------------

def _build(meta, n_feat, n_blk, n_scales):
    TL, NPL = meta["TL"], meta["NPL"]
    SH = NPL + P
    NTBL = NCORES * SH
    HALF = meta["HALF"]
    TLH = HALF // P
    calls, cols16 = meta["calls"], meta["cols16"]
    Kslab = meta["Kslab"]
    dt = mybir.dt.float32
    dtb = mybir.dt.bfloat16 if TBL_BF16 else dt
    nblocks = n_blk * n_scales

    nc = bass.Bass()

    featsT_p = nc.declare_dram_parameter("featsT", [n_feat, NPL], dt,
                                         isOutput=False)
    idx_p = [nc.declare_dram_parameter(f"idx{i}", [P, max(cols16[i], 1)],
                                       mybir.dt.int32, isOutput=False)
             for i in range(n_scales)]

    wshapes = {"w_in1": [n_feat, C], "w_int": [n_feat, C], "w_in2": [C, C]}
    wnames = ["w_in1", "w_int", "w_in2"]
    for k in range(nblocks):
        for nm in (f"fc1w{k}", f"fc2wa{k}", f"fc2wb{k}", f"linw{k}"):
            wnames.append(nm)
            wshapes[nm] = [C, C]
    gnames = ["g_in1", "b_in1", "g_in2", "b_in2", "g_int", "b_int"]
    for k in range(nblocks):
        gnames += [f"g_fc1{k}", f"b_fc1{k}", f"g_fc2{k}", f"b_fc2{k}",
                   f"g_lin{k}", f"b_lin{k}"]

    params = {}
    for nm in wnames:
        params[nm] = nc.declare_dram_parameter(nm, wshapes[nm], dt,
                                               isOutput=False)
    for nm in gnames:
        params[nm] = nc.declare_dram_parameter(nm, [P, 4 * C], dt,
                                               isOutput=False)

    out_p = nc.declare_dram_parameter("out", [P, NPL], dt, isOutput=True)

    fc1loc = [nc.dram_tensor(f"fc1loc{k}", [NPL + P, C], dtb)
              for k in range(nblocks)]
    tbl = [nc.dram_tensor(f"tbl{k}", [NTBL, C], dtb, addr_space="Shared")
           for k in range(nblocks)]

    from contextlib import ExitStack
    with ExitStack() as ctx:
        tc = ctx.enter_context(_TileContext(nc))
        const = ctx.enter_context(tc.tile_pool(name="const", bufs=1))
        big = ctx.enter_context(tc.tile_pool(name="big", bufs=1))
        wpool = ctx.enter_context(tc.tile_pool(name="wpool", bufs=2))
        work = ctx.enter_context(tc.tile_pool(name="work", bufs=3))
        gwork = ctx.enter_context(tc.tile_pool(name="gwork", bufs=4))
        slabp = ctx.enter_context(tc.tile_pool(name="slabp", bufs=3))
        idxp_pool = ctx.enter_context(tc.tile_pool(name="idxp", bufs=2))
        ps = ctx.enter_context(tc.tile_pool(name="ps", bufs=1, space="PSUM"))

        ident = const.tile([P, P], dt, tag="ident")
        make_identity(nc, ident[:])
        identb = const.tile([P, P], dtb, tag="identb")
        nc.vector.tensor_copy(identb[:], ident[:])
        eps_t = const.tile([P, 1], dt, tag="eps")
        nc.vector.memset(eps_t[:], EPS * P)   # bias for the 128*var sqrt
        zrow = const.tile([P, C], dtb, tag="zrow")
        nc.vector.memset(zrow[:], 0.0)

        feat = big.tile([P, NPL], dt, tag="feat")
        agg = big.tile([P, NPL], dtb, tag="agg")
        max_cols = max(max(cols16[i] for i in range(n_scales)), 1)
        SQP = float(P) ** 0.5

        def load_weights(mats, gnorms):
            sb = {}
            for j, nm in enumerate(mats):
                t = wpool.tile(wshapes[nm], dt, tag=f"wm{j}")
                nc.sync.dma_start(out=t[:], in_=params[nm][:])
                sb[nm] = t
            for j, nm in enumerate(gnorms):
                t = wpool.tile([P, 4 * C], dt, tag=f"wg{j}")
                nc.sync.dma_start(out=t[:], in_=params[nm][:])
                sb[nm] = t
            return sb

        def gn_batch(x3, T, g_t, b_t, out_ap, relu):
            """GroupNorm over C for T tiles at once.

            x3: [P, T, C] AP (PSUM); g_t/b_t: [P, 4C] tiled params;
            out_ap: [P, T*C].  Uses grouped bn_stats ([P,T,6] records of
            even/odd half-stats) and recombines mean/var manually.
            """
            st = gwork.tile([P, 4 * 6], dt, tag="st")
            st3 = st[:, :T * 6].rearrange("p (t s) -> p t s", s=6)
            for t in range(T):
                nc.vector.bn_stats(st[:, t * 6:(t + 1) * 6], x3[:, t, :])
            ms = gwork.tile([P, 4 * 4], dt, tag="ms")  # [msum|d|m2|vv]
            nc.vector.tensor_add(ms[:, 0:T], st3[:, :, 1], st3[:, :, 4])
            nc.vector.tensor_sub(ms[:, 4:4 + T], st3[:, :, 1], st3[:, :, 4])
            nc.vector.tensor_add(ms[:, 8:8 + T], st3[:, :, 2], st3[:, :, 5])
            dd = gwork.tile([P, 4], dt, tag="dd")
            nc.vector.tensor_mul(dd[:, :T], ms[:, 4:4 + T], ms[:, 4:4 + T])
            # 128*var = m2sum + 32*d^2
            nc.vector.scalar_tensor_tensor(ms[:, 12:12 + T], dd[:, :T],
                                           float(P // 4), ms[:, 8:8 + T],
                                           op0=ALU.mult, op1=ALU.add)
            sq = gwork.tile([P, 4], dt, tag="sq")
            nc.scalar.activation(sq[:, :T], ms[:, 12:12 + T], AFT.Sqrt,
                                 bias=eps_t[:], scale=1.0)
            rs = gwork.tile([P, 4], dt, tag="rs")
            nc.vector.reciprocal(rs[:, :T], sq[:, :T])
            nc.vector.tensor_scalar_mul(rs[:, :T], rs[:, :T], SQP)
            nm_ = gwork.tile([P, 4], dt, tag="nm")
            nc.vector.scalar_tensor_tensor(nm_[:, :T], ms[:, 0:T], -0.5,
                                           rs[:, :T], op0=ALU.mult,
                                           op1=ALU.mult)
            xc = gwork.tile([P, 4 * C], dt, tag="xc")
            for t in range(T):
                nc.scalar.activation(xc[:, t * C:(t + 1) * C], x3[:, t, :],
                                     AFT.Identity, bias=nm_[:, t:t + 1],
                                     scale=rs[:, t:t + 1])
            y = gwork.tile([P, 4 * C], dt, tag="y")
            nc.vector.tensor_mul(y[:, :T * C], xc[:, :T * C], g_t[:, :T * C])
            if relu:
                nc.vector.tensor_add(y[:, :T * C], y[:, :T * C],
                                     b_t[:, :T * C])
                nc.scalar.activation(out_ap, y[:, :T * C], AFT.Relu)
            else:
                nc.vector.tensor_add(out_ap, y[:, :T * C], b_t[:, :T * C])

        def transpose_grp(aps, ptag, stag, idn=None):
            T = len(aps)
            pdt = aps[0].dtype
            pt = ps.tile([P, 4 * C], pdt, tag=ptag)
            for j, ap in enumerate(aps):
                nc.tensor.transpose(pt[:, j * C:(j + 1) * C], ap,
                                    (idn or ident)[:])
            s = work.tile([P, 4 * C], dt, tag=stag)
            nc.any.tensor_copy(s[:, :T * C], pt[:, :T * C])
            return s

        def fcols(t0, T):
            return [feat[:, (t0 + j) * P:(t0 + j + 1) * P] for j in range(T)]

        def fc1_group(k, sbk, t0, T):
            fT = transpose_grp(fcols(t0, T), "tpn", "fTn")
            pm = ps.tile([P, 4 * C], dt, tag="mmn")
            for j in range(T):
                sl = slice(j * C, (j + 1) * C)
                nc.tensor.matmul(pm[:, sl], fT[:, sl], sbk[f"fc1w{k}"][:],
                                 start=True, stop=True)
            z = work.tile([P, 4 * C], dtb, tag="z")
            gn_batch(pm[:, :T * C].rearrange("p (t c) -> p t c", c=C), T,
                     sbk[f"g_fc1{k}"], sbk[f"b_fc1{k}"], z[:, :T * C],
                     relu=True)
            for j in range(T):
                t = t0 + j
                nc.sync.dma_start(out=fc1loc[k][t * P:(t + 1) * P, :],
                                  in_=z[:, j * C:(j + 1) * C])

        def fc2_group(k, sbk, t0, T):
            fT = transpose_grp(fcols(t0, T), "tpf", "fTs")
            aT = transpose_grp([agg[:, (t0 + j) * P:(t0 + j + 1) * P]
                                for j in range(T)], "tpa", "aTs", idn=identb)
            pm = ps.tile([P, 4 * C], dt, tag="mmf")
            for j in range(T):
                sl = slice(j * C, (j + 1) * C)
                nc.tensor.matmul(pm[:, sl], fT[:, sl], sbk[f"fc2wa{k}"][:],
                                 start=True, stop=False)
                nc.tensor.matmul(pm[:, sl], aT[:, sl], sbk[f"fc2wb{k}"][:],
                                 start=False, stop=True)
            h2 = work.tile([P, 4 * C], dt, tag="h2")
            gn_batch(pm[:, :T * C].rearrange("p (t c) -> p t c", c=C), T,
                     sbk[f"g_fc2{k}"], sbk[f"b_fc2{k}"], h2[:, :T * C],
                     relu=True)
            h2T = transpose_grp([h2[:, j * C:(j + 1) * C] for j in range(T)],
                                "tph", "h2Ts")
            pl = ps.tile([P, 4 * C], dt, tag="mml")
            for j in range(T):
                sl = slice(j * C, (j + 1) * C)
                nc.tensor.matmul(pl[:, sl], h2T[:, sl], sbk[f"linw{k}"][:],
                                 start=True, stop=True)
            h3 = work.tile([P, 4 * C], dt, tag="h3")
            gn_batch(pl[:, :T * C].rearrange("p (t c) -> p t c", c=C), T,
                     sbk[f"g_lin{k}"], sbk[f"b_lin{k}"], h3[:, :T * C],
                     relu=False)
            s2 = work.tile([P, 4 * C], dt, tag="s2")
            fsl = slice(t0 * P, (t0 + T) * P)
            nc.vector.tensor_add(s2[:, :T * C], h3[:, :T * C], feat[:, fsl])
            nc.scalar.activation(feat[:, fsl], s2[:, :T * C], AFT.Relu)

        def gather_tile(k, i, idx_t, t):
            (col, Wt) = calls[i][t]
            aslice = agg[:, t * C:(t + 1) * C]
            if Wt == 0:
                nc.vector.memset(aslice, 0.0)
                return
            slab = slabp.tile([P, Kslab * C], dtb, tag="slab")
            for kk in range(Wt):
                nc.gpsimd.indirect_dma_start(
                    out=slab[:, kk * C:(kk + 1) * C], out_offset=None,
                    in_=tbl[k][:, :],
                    in_offset=bass.IndirectOffsetOnAxis(
                        ap=idx_t[:, col + kk:col + kk + 1], axis=0),
                    compute_op=ALU.bypass)
            W = Wt
            if W == 1:
                nc.vector.tensor_copy(aslice, slab[:, :C])
            while W > 1:
                h = W // 2
                off2 = (W + 1) // 2
                out_ap = aslice if W == 2 else slab[:, :h * C]
                nc.vector.tensor_max(out_ap, slab[:, :h * C],
                                     slab[:, off2 * C:(off2 + h) * C])
                W = (W + 1) // 2

        def emit_allgather_a(k):
            nc.gpsimd.collective_compute(
                "AllGather", ALU.bypass,
                replica_groups=[list(range(NCORES))],
                ins=[fc1loc[k][:HALF]], outs=[tbl[k][:NCORES * HALF]])

        def emit_allgather_b(k):
            nc.sync.dma_start(out=fc1loc[k][NPL:NPL + P, :], in_=zrow[:])
            nc.gpsimd.collective_compute(
                "AllGather", ALU.bypass,
                replica_groups=[list(range(NCORES))],
                ins=[fc1loc[k][HALF:]], outs=[tbl[k][NCORES * HALF:]])

        def input_group(sb_in, t0, T):
            fs_t = work.tile([n_feat, 4 * P], dt, tag="fs")
            nc.sync.dma_start(out=fs_t[:, :T * P],
                              in_=featsT_p[:, t0 * P:(t0 + T) * P])
            p1 = ps.tile([P, 4 * C], dt, tag="mmf")
            for j in range(T):
                nc.tensor.matmul(p1[:, j * C:(j + 1) * C],
                                 fs_t[:, j * P:(j + 1) * P],
                                 sb_in["w_in1"][:], start=True, stop=True)
            h1 = work.tile([P, 4 * C], dt, tag="h2")
            gn_batch(p1[:, :T * C].rearrange("p (t c) -> p t c", c=C), T,
                     sb_in["g_in1"], sb_in["b_in1"], h1[:, :T * C], relu=True)
            h1T = transpose_grp([h1[:, j * C:(j + 1) * C] for j in range(T)],
                                "tph", "h2Ts")
            p2 = ps.tile([P, 4 * C], dt, tag="mml")
            for j in range(T):
                nc.tensor.matmul(p2[:, j * C:(j + 1) * C],
                                 h1T[:, j * C:(j + 1) * C],
                                 sb_in["w_in2"][:], start=True, stop=True)
            o2 = work.tile([P, 4 * C], dt, tag="h3")
            gn_batch(p2[:, :T * C].rearrange("p (t c) -> p t c", c=C), T,
                     sb_in["g_in2"], sb_in["b_in2"], o2[:, :T * C],
                     relu=False)
            p3 = ps.tile([P, 4 * C], dt, tag="mmn")
            for j in range(T):
                nc.tensor.matmul(p3[:, j * C:(j + 1) * C],
                                 fs_t[:, j * P:(j + 1) * P],
                                 sb_in["w_int"][:], start=True, stop=True)
            o3 = work.tile([P, 4 * C], dt, tag="o3")
            gn_batch(p3[:, :T * C].rearrange("p (t c) -> p t c", c=C), T,
                     sb_in["g_int"], sb_in["b_int"], o3[:, :T * C],
                     relu=False)
            s = work.tile([P, 4 * C], dt, tag="s2")
            nc.vector.tensor_add(s[:, :T * C], o2[:, :T * C], o3[:, :T * C])
            nc.scalar.activation(feat[:, t0 * P:(t0 + T) * P], s[:, :T * C],
                                 AFT.Relu)

        def wnames_of(k):
            return ([f"fc1w{k}", f"fc2wa{k}", f"fc2wb{k}", f"linw{k}"],
                    [f"g_fc1{k}", f"b_fc1{k}", f"g_fc2{k}", f"b_fc2{k}",
                     f"g_lin{k}", f"b_lin{k}"])

        def load_idx(i):
            idx_t = idxp_pool.tile([P, max_cols], mybir.dt.int32, tag="idx")
            if cols16[i] > 0:
                nc.sync.dma_start(out=idx_t[:, :cols16[i]], in_=idx_p[i][:])
            return idx_t

        # tile groups of up to 4; TLH must be a group boundary
        groups = []
        t0 = 0
        while t0 < TL:
            T = min(4, TL - t0, (TLH - t0) if t0 < TLH else TL - t0)
            groups.append((t0, T))
            t0 += T
        # Pool-stream position at which AllGather-a(k+1) is emitted: late
        # enough (~75% of the block) that the first-half fc1 tiles it ships
        # are already computed, so the Pool engine does not stall there.
        agath = min((t0 + T for (t0, T) in groups
                     if t0 + T >= (TLH + TL) // 2), default=TLH)

        # ---------------- bootstrap: input block + block 0 fc1 -----------
        sb_in = load_weights(["w_in1", "w_int", "w_in2"],
                             ["g_in1", "b_in1", "g_in2", "b_in2",
                              "g_int", "b_int"])
        sbs = {0: load_weights(*wnames_of(0))}
        idxs = {0: load_idx(0)}
        for (t0, T) in groups:
            input_group(sb_in, t0, T)
            fc1_group(0, sbs[0], t0, T)
            if t0 + T == TLH:
                emit_allgather_a(0)
        emit_allgather_b(0)

        # ---------------- pipelined aggregation blocks -------------------
        for k in range(nblocks):
            i = k % n_scales
            if k + 1 < nblocks:
                sbs[k + 1] = load_weights(*wnames_of(k + 1))
                idxs[k + 1] = load_idx((k + 1) % n_scales)
            for (t0, T) in groups:
                for t in range(t0, t0 + T):
                    gather_tile(k, i, idxs[k], t)
                fc2_group(k, sbs[k], t0, T)
                if k + 1 < nblocks:
                    fc1_group(k + 1, sbs[k + 1], t0, T)
                    if t0 + T == agath:
                        emit_allgather_a(k + 1)
            if k + 1 < nblocks:
                emit_allgather_b(k + 1)
            del sbs[k], idxs[k]

        nc.sync.dma_start(out=out_p[:], in_=feat[:])

    _legalize_waits(nc)
    return nc


# ---------------------------------------------------------------------------
# input maps / output assembly
# ---------------------------------------------------------------------------

def _make_in_maps(inputs, meta, n_feat, n_blk, n_scales):
    feats = np.asarray(inputs["feats"], np.float32)
    NPL = meta["NPL"]
    order = meta["order"]
    nblocks = n_blk * n_scales

    shared = {
        "w_in1": np.asarray(inputs["in_w1"], np.float32),
        "w_int": np.asarray(inputs["in_wt"], np.float32),
        "w_in2": np.asarray(inputs["in_w2"], np.float32),
        "g_in1": _bc(inputs["in_g1"]), "b_in1": _bc(inputs["in_b1"]),
        "g_in2": _bc(inputs["in_g2"]), "b_in2": _bc(inputs["in_b2"]),
        "g_int": _bc(inputs["in_gt"]), "b_int": _bc(inputs["in_bt"]),
    }
    fc2w = np.asarray(inputs["fc2_w"], np.float32)
    for k in range(nblocks):
        shared[f"fc1w{k}"] = np.ascontiguousarray(
            np.asarray(inputs["fc1_w"], np.float32)[k])
        shared[f"fc2wa{k}"] = np.ascontiguousarray(fc2w[k, :C])
        shared[f"fc2wb{k}"] = np.ascontiguousarray(fc2w[k, C:])
        shared[f"linw{k}"] = np.ascontiguousarray(
            np.asarray(inputs["lin_w"], np.float32)[k])
        shared[f"g_fc1{k}"] = _bc(inputs["fc1_g"][k])
        shared[f"b_fc1{k}"] = _bc(inputs["fc1_b"][k])
        shared[f"g_fc2{k}"] = _bc(inputs["fc2_g"][k])
        shared[f"b_fc2{k}"] = _bc(inputs["fc2_b"][k])
        shared[f"g_lin{k}"] = _bc(inputs["lin_g"][k])
        shared[f"b_lin{k}"] = _bc(inputs["lin_b"][k])

    in_maps = []
    for c in range(NCORES):
        m = dict(shared)
        ft = np.zeros((n_feat, NPL), np.float32)
        valid = order[c] >= 0
        ft[:, valid] = feats[order[c][valid]].T
        m["featsT"] = np.ascontiguousarray(ft)
        for i in range(n_scales):
            m[f"idx{i}"] = meta["idx"][i][c]
        in_maps.append(m)
    return in_maps


def _assemble(outs, meta, n_nodes):
    TL, NPL = meta["TL"], meta["NPL"]
    order = meta["order"]
    full = np.zeros((n_nodes, C), np.float32)
    for c in range(NCORES):
        o = np.asarray(outs[c]["out"])           # [P, NPL]
        rows = o.reshape(P, TL, C).transpose(1, 0, 2).reshape(NPL, C)
        valid = order[c] >= 0
        full[order[c][valid]] = rows[valid]
    return full


# ---------------------------------------------------------------------------
# entry points
# ---------------------------------------------------------------------------

def forward(inputs, geom=None, runner="hw", trace=False):
    """Run the kernel. runner: 'hw' (Trainium via SPMD) or 'sim' (CoreSim)."""
    g = dict(FULL_GEOM)
    if geom:
        g.update(geom)
    n_nodes, n_feat = g["n_nodes"], g["n_feat"]
    n_blk, n_scales = g["n_blk"], g["n_scales"]

    meta = _host_prep(inputs["u"], inputs["v"], n_nodes)
    nc = _build(meta, n_feat, n_blk, n_scales)
    in_maps = _make_in_maps(inputs, meta, n_feat, n_blk, n_scales)

    info = {}
    if runner == "sim":
        from concourse.bass_interp import MultiCoreSim
        sim = MultiCoreSim(nc, NCORES)
        for c in range(NCORES):
            for k_, v_ in in_maps[c].items():
                sim.cores[c].tensor(k_)[:] = v_
        sim.simulate()
        outs = [{"out": sim.cores[c].tensor("out").copy()}
                for c in range(NCORES)]
    else:
        from concourse.bass_utils import run_bass_kernel_spmd
        res = run_bass_kernel_spmd(nc, in_maps, list(range(NCORES)),
                                   trace=trace)
        outs = res.results
        info["exec_time_ns"] = res.exec_time_ns
        info["profile_json"] = res.profile_json

    return _assemble(outs, meta, n_nodes), info


def forward_timed(inputs, geom=None, iters=3):
    """Like forward(runner='hw') but keeps the jitted SPMD executable and
    times repeated executions."""
    import time as _time

    import jax
    from jax.sharding import Mesh, PartitionSpec
    from jax.experimental.shard_map import shard_map
    from concourse import bass2jax

    g = dict(FULL_GEOM)
    if geom:
        g.update(geom)
    n_nodes, n_feat = g["n_nodes"], g["n_feat"]
    n_blk, n_scales = g["n_blk"], g["n_scales"]

    meta = _host_prep(inputs["u"], inputs["v"], n_nodes)
    nc = _build(meta, n_feat, n_blk, n_scales)
    in_maps = _make_in_maps(inputs, meta, n_feat, n_blk, n_scales)

    bass2jax.install_neuronx_cc_hook()
    nc.finalize()

    partition_name = (nc.partition_id_tensor.name
                      if nc.partition_id_tensor else None)
    import concourse.mybir as mb
    in_names, out_names, out_avals, zero_outs = [], [], [], []
    for alloc in nc.m.functions[0].allocations:
        if not isinstance(alloc, mb.MemoryLocationSet):
            continue
        name = alloc.memorylocations[0].name
        if alloc.kind == "ExternalInput":
            if name != partition_name:
                in_names.append(name)
        elif alloc.kind == "ExternalOutput":
            shape = tuple(alloc.tensor_shape)
            dtype = mb.dt.np(alloc.dtype)
            out_names.append(name)
            out_avals.append(jax.core.ShapedArray(shape, dtype))
            zero_outs.append(np.zeros(shape, dtype))
    n_params = len(in_names)
    n_outs = len(out_avals)
    in_names = in_names + out_names
    if partition_name is not None:
        in_names.append(partition_name)
    donate = tuple(range(n_params, n_params + n_outs))

    def _body(*args):
        operands = list(args)
        if partition_name is not None:
            operands.append(bass2jax.partition_id_tensor())
        outs = bass2jax._bass_exec_p.bind(
            *operands, out_avals=tuple(out_avals), in_names=tuple(in_names),
            out_names=tuple(out_names), lowering_input_output_aliases=(),
            sim_require_finite=True, sim_require_nnan=True, nc=nc)
        return tuple(outs)

    devices = jax.devices()[:NCORES]
    mesh = Mesh(np.asarray(devices), ("core",))
    sharded = jax.jit(
        shard_map(_body, mesh=mesh,
                  in_specs=(PartitionSpec("core"),) * (n_params + n_outs),
                  out_specs=(PartitionSpec("core"),) * n_outs,
                  check_rep=False),
        donate_argnums=donate, keep_unused=True)

    from jax.sharding import NamedSharding
    shard = NamedSharding(mesh, PartitionSpec("core"))
    concat_in = [jax.device_put(
        np.concatenate([np.asarray(in_maps[c][nm]) for c in range(NCORES)],
                       axis=0), shard) for nm in in_names[:n_params]]
    staged_zeros = [[jax.device_put(
        np.zeros((NCORES * z.shape[0], *z.shape[1:]), z.dtype), shard)
        for z in zero_outs] for _ in range(iters)]
    jax.block_until_ready(concat_in)
    jax.block_until_ready(staged_zeros)
    times = []
    out_arrs = None
    for it in range(iters):
        t0 = _time.time()
        out_arrs = sharded(*concat_in, *staged_zeros[it])
        jax.block_until_ready(out_arrs)
        times.append(time := _time.time() - t0)
        print(f"  iter {it}: {time*1e3:.2f} ms wall")
    outs = [{nm: np.asarray(out_arrs[j]).reshape(NCORES,
                                                 *out_avals[j].shape)[c]
             for j, nm in enumerate(out_names)} for c in range(NCORES)]
    full = _assemble(outs, meta, n_nodes)
    return full, dict(times=times, best_wall_s=min(times[1:])
                      if len(times) > 1 else times[0])


def kernel(**inputs) -> np.ndarray:
    out, _ = forward(inputs)
    return out


# revision 34
# speedup vs baseline: 1.1880x; 1.1880x over previous
"""Distributed Trainium2 (Bass/Tile) kernel for the GNN message-passing problem.

Strategy (8 NeuronCores, SPMD):
  * Nodes are partitioned across the 8 cores (12500 each). Within a core,
    local nodes are sorted by total in-degree desc, then each 1024-node
    band is re-sorted by the cross-scale degree difference, so every
    128-node tile is degree-homogeneous for BOTH edge scales -> the
    round-based gather below pads few slots (the per-tile round count is
    the tile's max per-scale in-degree).
  * Small weight tensors are replicated to every core.
  * Per aggregation block: each core computes fc_1 features for its local
    nodes, the shards are exchanged with an AllGather into a replicated
    [N_tbl, 128] bf16 DRAM table, and the scatter_max is computed locally:
    round r gathers the r-th incoming edge of every local node with one
    [128,1]-offset indirect DMA (pad slots point at zeroed table rows),
    and a halving tree of tensor_max ops reduces the rounds into the agg
    tile.  relu(...) >= 0 makes the zero rows the identity of the max.
  * The gather DMAs are the serial resource (one SWDGE descriptor-gen per
    round, ~1.1 us each on the Pool engine).  To keep Pool saturated, the
    per-tile work of three pipeline stages is INTERLEAVED in emission
    order: gather(k,t) ; fc2(k,t) ; fc1(k+1,t) — so the PE/DVE/ACT work
    of block k's tail and block k+1's fc1 runs in the shadow of block k's
    remaining gather rounds, and the next AllGather fires as soon as the
    last fc1 tile is written.

Host-side prep only touches index tensors / layout (graph partitioning),
never the float data.
"""

import sys

for _p in ("/opt/trn_rl_repo", "/root/.axon_site/_ro/trn_rl_repo"):
    if _p not in sys.path:
        sys.path.append(_p)

import numpy as np

import concourse.bass as bass
import concourse.tile as tile
from concourse import mybir
from concourse.masks import make_identity
from concourse.tile import ScopedClock


class _TileContext(tile.TileContext):
    """TileContext whose tail drain carries at most one sync wait.

    The walrus build in this container rejects TPB_CTRL instructions with
    more than a couple of sync waits ("Too many sync wait commands"), and
    the stock tail drain waits on every live semaphore at once.  Split the
    waits onto single-wait NOPs in front of the drain instead.
    """

    def _drain_and_barrier(self, tick_clock, wait_clock):
        nc = self.nc
        probe = nc.sync.nop(nofuse=True)
        wait_clock.add_sem_waits(probe.ins,
                                 ScopedClock({None: tick_clock.global_clock}))
        si = probe.ins.sync_info
        waits = list(si.on_wait or []) if si else []
        upd = list(si.on_update or []) if si else []
        probe.ins.sync_info = mybir.SyncInfo(on_wait=waits[:1], on_update=upd)
        for w in waits[1:]:
            n = nc.sync.nop(nofuse=True)
            n.ins.sync_info = mybir.SyncInfo(on_wait=[w], on_update=[])
        nc.sync.drain()
        nc.all_engine_barrier()
        assert self.sems is not None
        popped = nc._tile_sem_poison_stack.pop()
        assert popped is self._sem_poison
        nc.clear_and_free_semaphores(list(self.sems.allocated().values()))
        nc.all_engine_barrier()

P = 128           # partitions / tile rows
C = 128           # channels (N_MAP)
NCORES = 8
EPS = 1e-5

AFT = mybir.ActivationFunctionType
ALU = mybir.AluOpType

FULL_GEOM = dict(n_nodes=100000, n_feat=22, n_scales=2, n_blk=2)

TBL_BF16 = True  # exchange/gather the fc1 table in bf16


def _host_prep(u, v, n_nodes):
    """Compute per-core node ordering and per-scale gather index arrays.

    Returns dict with:
      order : [NCORES, NPL] global node id per local slot (or -1 for pad)
      calls : per-scale list of per-tile (col, K)
      cols16: per-scale total index columns
      idx   : per-scale list of per-core [P, cols] int32 gather indices
      TL, NPL, Kslab (max per-tile round count), HALF
    """
    n_scales = u.shape[0]
    nloc = n_nodes // NCORES
    TL = (nloc + P - 1) // P
    NPL = TL * P
    SH = NPL + P          # shard rows in the table (incl. P zero rows)
    HALF = (TL // 2) * P  # first-half slots exchanged by the early AllGather

    u = [np.asarray(u[i]).astype(np.int64) for i in range(n_scales)]
    v = [np.asarray(v[i]).astype(np.int64) for i in range(n_scales)]

    deg = np.zeros((n_scales, n_nodes), np.int64)
    for i in range(n_scales):
        deg[i] = np.bincount(v[i], minlength=n_nodes)
    score = deg.max(axis=0)
    score2 = deg[0] - deg[1] if n_scales > 1 else deg[0]

    order = np.full((NCORES, NPL), -1, np.int64)
    slot = np.zeros(n_nodes, np.int64)
    for c in range(NCORES):
        ids = np.arange(c * nloc, min((c + 1) * nloc, n_nodes))
        o = ids[np.argsort(-score[ids], kind="stable")]
        parts = []
        for b in range(0, len(o), 1024):
            blk = o[b:b + 1024]
            parts.append(blk[np.argsort(-score2[blk], kind="stable")])
        o = np.concatenate(parts) if parts else o
        order[c, : len(o)] = o
        slot[o] = np.arange(len(o))

    core_of = np.arange(n_nodes) // nloc
    np.minimum(core_of, NCORES - 1, out=core_of)

    calls_all, cols_all, idx_all = [], [], []
    for i in range(n_scales):
        ui, vi = u[i], v[i]
        dst_core = core_of[vi]
        dst_slot = slot[vi]
        s_u = slot[ui]
        c_u = core_of[ui]
        src_tbl = np.where(
            s_u < HALF, c_u * HALF + s_u,
            NCORES * HALF + c_u * (SH - HALF) + (s_u - HALF)).astype(np.int64)

        per_core = []
        K = np.zeros((NCORES, TL), np.int64)
        for c in range(NCORES):
            sel = np.nonzero(dst_core == c)[0]
            ls = dst_slot[sel]
            srt = np.argsort(ls, kind="stable")
            ls_s = ls[srt]
            first = np.searchsorted(ls_s, ls_s, side="left")
            rank = np.arange(len(ls_s)) - first
            per_core.append((sel[srt], ls_s, rank))
            cnt = np.bincount(ls, minlength=NPL)
            K[c] = cnt.reshape(TL, P).max(axis=1)

        Kt = K.max(axis=0)                       # [TL] shared program shape
        calls = []
        col = 0
        col_of_tile = np.zeros(TL, np.int64)
        for t in range(TL):
            kk = int(Kt[t])
            calls.append((col, kk))
            col_of_tile[t] = col
            col += kk
        cols = col

        per_core_idx = []
        for c in range(NCORES):
            eidx, ls_s, rank = per_core[c]
            pad0 = NCORES * HALF + (NPL - HALF)
            arr = np.broadcast_to(
                (pad0 + np.arange(P, dtype=np.int32))[:, None],
                (P, max(cols, 1))).copy()
            t_e = ls_s // P
            p_e = ls_s % P
            arr[p_e, col_of_tile[t_e] + rank] = src_tbl[eidx].astype(np.int32)
            per_core_idx.append(arr)

        calls_all.append(calls)
        cols_all.append(cols)
        idx_all.append(per_core_idx)

    Kslab = max((kk for cl in calls_all for (_, kk) in cl), default=1)
    return dict(order=order, calls=calls_all, cols16=cols_all, idx=idx_all,
                TL=TL, NPL=NPL, nloc=nloc, Kslab=Kslab, HALF=HALF)


def _legalize_waits(nc, maxw=1):
    """Split multi-wait instructions into single-wait NOPs + the instruction."""
    f = nc.m.functions[0]
    n_split = 0
    for blk in f.blocks:
        insts = list(blk.instructions)
        if not any(i.sync_info and i.sync_info.on_wait
                   and len(i.sync_info.on_wait) > maxw for i in insts):
            continue
        new = []
        for inst in insts:
            si = inst.sync_info
            waits = list(si.on_wait) if si and si.on_wait else []
            if len(waits) > maxw:
                for j, w in enumerate(waits[:-maxw]):
                    nop = mybir.InstNoOp(
                        name=f"{inst.name}-sw{j}", engine=inst.engine,
                        ins=[], outs=[],
                        sync_info=mybir.SyncInfo(on_wait=[w], on_update=[]))
                    nc.register_instruction(nop, overwrite=True)
                    new.append(nop)
                    n_split += 1
                inst.sync_info = mybir.SyncInfo(
                    on_wait=waits[-maxw:], on_update=list(si.on_update or []))
            new.append(inst)
        blk.instructions = new
    return n_split


def _bc(x):
    """broadcast a [C] vector to a [P, 4*C] f32 tile (4x tiled for the
    4-tile batched GroupNorm)."""
    return np.ascontiguousarray(np.tile(np.broadcast_to(
        np.asarray(x, np.float32).reshape(1, C), (P, C)), (1, 4)))


# ---------------------------------------------------------------------------
# program builder
# ---------------------------------------------------------------------------

def _build(meta, n_feat, n_blk, n_scales):
    TL, NPL = meta["TL"], meta["NPL"]
    SH = NPL + P
    NTBL = NCORES * SH
    HALF = meta["HALF"]
    TLH = HALF // P
    calls, cols16 = meta["calls"], meta["cols16"]
    Kslab = meta["Kslab"]
    dt = mybir.dt.float32
    dtb = mybir.dt.bfloat16 if TBL_BF16 else dt
    nblocks = n_blk * n_scales

    nc = bass.Bass()

    featsT_p = nc.declare_dram_parameter("featsT", [n_feat, NPL], dt,
                                         isOutput=False)
    idx_p = [nc.declare_dram_parameter(f"idx{i}", [P, max(cols16[i], 1)],
                                       mybir.dt.int32, isOutput=False)
             for i in range(n_scales)]

    wshapes = {"w_in1": [n_feat, C], "w_int": [n_feat, C], "w_in2": [C, C]}
    wnames = ["w_in1", "w_int", "w_in2"]
    for k in range(nblocks):
        for nm in (f"fc1w{k}", f"fc2wa{k}", f"fc2wb{k}", f"linw{k}"):
            wnames.append(nm)
            wshapes[nm] = [C, C]
    gnames = ["g_in1", "b_in1", "g_in2", "b_in2", "g_int", "b_int"]
    for k in range(nblocks):
        gnames += [f"g_fc1{k}", f"b_fc1{k}", f"g_fc2{k}", f"b_fc2{k}",
                   f"g_lin{k}", f"b_lin{k}"]

    params = {}
    for nm in wnames:
        params[nm] = nc.declare_dram_parameter(nm, wshapes[nm], dt,
                                               isOutput=False)
    for nm in gnames:
        params[nm] = nc.declare_dram_parameter(nm, [P, 4 * C], dt,
                                               isOutput=False)

    out_p = nc.declare_dram_parameter("out", [P, NPL], dt, isOutput=True)

    fc1loc = [nc.dram_tensor(f"fc1loc{k}", [NPL + P, C], dtb)
              for k in range(nblocks)]
    tbl = [nc.dram_tensor(f"tbl{k}", [NTBL, C], dtb, addr_space="Shared")
           for k in range(nblocks)]

    from contextlib import ExitStack
    with ExitStack() as ctx:
        tc = ctx.enter_context(_TileContext(nc))
        const = ctx.enter_context(tc.tile_pool(name="const", bufs=1))
        big = ctx.enter_context(tc.tile_pool(name="big", bufs=1))
        wpool = ctx.enter_context(tc.tile_pool(name="wpool", bufs=2))
        work = ctx.enter_context(tc.tile_pool(name="work", bufs=2))
        gwork = ctx.enter_context(tc.tile_pool(name="gwork", bufs=4))
        slabp = ctx.enter_context(tc.tile_pool(name="slabp", bufs=5))
        idxp_pool = ctx.enter_context(tc.tile_pool(name="idxp", bufs=2))
        ps = ctx.enter_context(tc.tile_pool(name="ps", bufs=1, space="PSUM"))

        ident = const.tile([P, P], dt, tag="ident")
        make_identity(nc, ident[:])
        identb = const.tile([P, P], dtb, tag="identb")
        nc.vector.tensor_copy(identb[:], ident[:])
        eps_t = const.tile([P, 1], dt, tag="eps")
        nc.vector.memset(eps_t[:], EPS * P)   # bias for the 128*var sqrt
        zrow = const.tile([P, C], dtb, tag="zrow")
        nc.vector.memset(zrow[:], 0.0)

        feat = big.tile([P, NPL], dt, tag="feat")
        agg = big.tile([P, NPL], dtb, tag="agg")
        max_cols = max(max(cols16[i] for i in range(n_scales)), 1)
        SQP = float(P) ** 0.5

        def load_weights(mats, gnorms):
            sb = {}
            for j, nm in enumerate(mats):
                t = wpool.tile(wshapes[nm], dt, tag=f"wm{j}")
                nc.sync.dma_start(out=t[:], in_=params[nm][:])
                sb[nm] = t
            for j, nm in enumerate(gnorms):
                t = wpool.tile([P, 4 * C], dt, tag=f"wg{j}")
                nc.sync.dma_start(out=t[:], in_=params[nm][:])
                sb[nm] = t
            return sb

        def gn_batch(x3, T, g_t, b_t, out_ap, relu):
            """GroupNorm over C for T tiles at once.

            x3: [P, T, C] AP (PSUM); g_t/b_t: [P, 4C] tiled params;
            out_ap: [P, T*C].  Uses grouped bn_stats ([P,T,6] records of
            even/odd half-stats) and recombines mean/var manually.
            """
            st = gwork.tile([P, 4 * 6], dt, tag="st")
            st3 = st[:, :T * 6].rearrange("p (t s) -> p t s", s=6)
            for t in range(T):
                nc.vector.bn_stats(st[:, t * 6:(t + 1) * 6], x3[:, t, :])
            ms = gwork.tile([P, 4 * 4], dt, tag="ms")  # [msum|d|m2|vv]
            nc.vector.tensor_add(ms[:, 0:T], st3[:, :, 1], st3[:, :, 4])
            nc.vector.tensor_sub(ms[:, 4:4 + T], st3[:, :, 1], st3[:, :, 4])
            nc.vector.tensor_add(ms[:, 8:8 + T], st3[:, :, 2], st3[:, :, 5])
            dd = gwork.tile([P, 4], dt, tag="dd")
            nc.vector.tensor_mul(dd[:, :T], ms[:, 4:4 + T], ms[:, 4:4 + T])
            # 128*var = m2sum + 32*d^2
            nc.vector.scalar_tensor_tensor(ms[:, 12:12 + T], dd[:, :T],
                                           float(P // 4), ms[:, 8:8 + T],
                                           op0=ALU.mult, op1=ALU.add)
            sq = gwork.tile([P, 4], dt, tag="sq")
            nc.scalar.activation(sq[:, :T], ms[:, 12:12 + T], AFT.Sqrt,
                                 bias=eps_t[:], scale=1.0)
            rs = gwork.tile([P, 4], dt, tag="rs")
            nc.vector.reciprocal(rs[:, :T], sq[:, :T])
            nc.vector.tensor_scalar_mul(rs[:, :T], rs[:, :T], SQP)
            nm_ = gwork.tile([P, 4], dt, tag="nm")
            nc.vector.scalar_tensor_tensor(nm_[:, :T], ms[:, 0:T], -0.5,
                                           rs[:, :T], op0=ALU.mult,
                                           op1=ALU.mult)
            xc = gwork.tile([P, 4 * C], dt, tag="xc")
            for t in range(T):
                nc.scalar.activation(xc[:, t * C:(t + 1) * C], x3[:, t, :],
                                     AFT.Identity, bias=nm_[:, t:t + 1],
                                     scale=rs[:, t:t + 1])
            y = gwork.tile([P, 4 * C], dt, tag="y")
            nc.vector.tensor_mul(y[:, :T * C], xc[:, :T * C], g_t[:, :T * C])
            if relu:
                nc.vector.tensor_add(y[:, :T * C], y[:, :T * C],
                                     b_t[:, :T * C])
                nc.scalar.activation(out_ap, y[:, :T * C], AFT.Relu)
            else:
                nc.vector.tensor_add(out_ap, y[:, :T * C], b_t[:, :T * C])

        def transpose_grp(aps, ptag, stag, idn=None):
            T = len(aps)
            pdt = aps[0].dtype
            pt = ps.tile([P, 4 * C], pdt, tag=ptag)
            for j, ap in enumerate(aps):
                nc.tensor.transpose(pt[:, j * C:(j + 1) * C], ap,
                                    (idn or ident)[:])
            s = work.tile([P, 4 * C], dt, tag=stag)
            nc.any.tensor_copy(s[:, :T * C], pt[:, :T * C])
            return s

        def fcols(t0, T):
            return [feat[:, (t0 + j) * P:(t0 + j + 1) * P] for j in range(T)]

        def fc1_group(k, sbk, t0, T):
            fT = transpose_grp(fcols(t0, T), "tpn", "fTn")
            pm = ps.tile([P, 4 * C], dt, tag="mmn")
            for j in range(T):
                sl = slice(j * C, (j + 1) * C)
                nc.tensor.matmul(pm[:, sl], fT[:, sl], sbk[f"fc1w{k}"][:],
                                 start=True, stop=True)
            z = work.tile([P, 4 * C], dtb, tag="z")
            gn_batch(pm[:, :T * C].rearrange("p (t c) -> p t c", c=C), T,
                     sbk[f"g_fc1{k}"], sbk[f"b_fc1{k}"], z[:, :T * C],
                     relu=True)
            for j in range(T):
                t = t0 + j
                nc.sync.dma_start(out=fc1loc[k][t * P:(t + 1) * P, :],
                                  in_=z[:, j * C:(j + 1) * C])

        def fc2_group(k, sbk, t0, T):
            fT = transpose_grp(fcols(t0, T), "tpf", "fTs")
            aT = transpose_grp([agg[:, (t0 + j) * P:(t0 + j + 1) * P]
                                for j in range(T)], "tpa", "aTs", idn=identb)
            pm = ps.tile([P, 4 * C], dt, tag="mmf")
            for j in range(T):
                sl = slice(j * C, (j + 1) * C)
                nc.tensor.matmul(pm[:, sl], fT[:, sl], sbk[f"fc2wa{k}"][:],
                                 start=True, stop=False)
                nc.tensor.matmul(pm[:, sl], aT[:, sl], sbk[f"fc2wb{k}"][:],
                                 start=False, stop=True)
            h2 = work.tile([P, 4 * C], dt, tag="h2")
            gn_batch(pm[:, :T * C].rearrange("p (t c) -> p t c", c=C), T,
                     sbk[f"g_fc2{k}"], sbk[f"b_fc2{k}"], h2[:, :T * C],
                     relu=True)
            h2T = transpose_grp([h2[:, j * C:(j + 1) * C] for j in range(T)],
                                "tph", "h2Ts")
            pl = ps.tile([P, 4 * C], dt, tag="mml")
            for j in range(T):
                sl = slice(j * C, (j + 1) * C)
                nc.tensor.matmul(pl[:, sl], h2T[:, sl], sbk[f"linw{k}"][:],
                                 start=True, stop=True)
            h3 = work.tile([P, 4 * C], dt, tag="h3")
            gn_batch(pl[:, :T * C].rearrange("p (t c) -> p t c", c=C), T,
                     sbk[f"g_lin{k}"], sbk[f"b_lin{k}"], h3[:, :T * C],
                     relu=False)
            s2 = work.tile([P, 4 * C], dt, tag="s2")
            fsl = slice(t0 * P, (t0 + T) * P)
            nc.vector.tensor_add(s2[:, :T * C], h3[:, :T * C], feat[:, fsl])
            nc.scalar.activation(feat[:, fsl], s2[:, :T * C], AFT.Relu)

        def gather_tile(k, i, idx_t, t):
            (col, Wt) = calls[i][t]
            aslice = agg[:, t * C:(t + 1) * C]
            if Wt == 0:
                nc.vector.memset(aslice, 0.0)
                return
            slab = slabp.tile([P, Kslab * C], dtb, tag="slab")
            for kk in range(Wt):
                nc.gpsimd.indirect_dma_start(
                    out=slab[:, kk * C:(kk + 1) * C], out_offset=None,
                    in_=tbl[k][:, :],
                    in_offset=bass.IndirectOffsetOnAxis(
                        ap=idx_t[:, col + kk:col + kk + 1], axis=0),
                    compute_op=ALU.bypass)
            W = Wt
            if W == 1:
                nc.vector.tensor_copy(aslice, slab[:, :C])
            while W > 1:
                h = W // 2
                off2 = (W + 1) // 2
                out_ap = aslice if W == 2 else slab[:, :h * C]
                nc.vector.tensor_max(out_ap, slab[:, :h * C],
                                     slab[:, off2 * C:(off2 + h) * C])
                W = (W + 1) // 2

        def emit_allgather_a(k):
            nc.gpsimd.collective_compute(
                "AllGather", ALU.bypass,
                replica_groups=[list(range(NCORES))],
                ins=[fc1loc[k][:HALF]], outs=[tbl[k][:NCORES * HALF]])

        def emit_allgather_b(k):
            nc.sync.dma_start(out=fc1loc[k][NPL:NPL + P, :], in_=zrow[:])
            nc.gpsimd.collective_compute(
                "AllGather", ALU.bypass,
                replica_groups=[list(range(NCORES))],
                ins=[fc1loc[k][HALF:]], outs=[tbl[k][NCORES * HALF:]])

        def input_group(sb_in, t0, T):
            fs_t = work.tile([n_feat, 4 * P], dt, tag="fs")
            nc.sync.dma_start(out=fs_t[:, :T * P],
                              in_=featsT_p[:, t0 * P:(t0 + T) * P])
            p1 = ps.tile([P, 4 * C], dt, tag="mmf")
            for j in range(T):
                nc.tensor.matmul(p1[:, j * C:(j + 1) * C],
                                 fs_t[:, j * P:(j + 1) * P],
                                 sb_in["w_in1"][:], start=True, stop=True)
            h1 = work.tile([P, 4 * C], dt, tag="h2")
            gn_batch(p1[:, :T * C].rearrange("p (t c) -> p t c", c=C), T,
                     sb_in["g_in1"], sb_in["b_in1"], h1[:, :T * C], relu=True)
            h1T = transpose_grp([h1[:, j * C:(j + 1) * C] for j in range(T)],
                                "tph", "h2Ts")
            p2 = ps.tile([P, 4 * C], dt, tag="mml")
            for j in range(T):
                nc.tensor.matmul(p2[:, j * C:(j + 1) * C],
                                 h1T[:, j * C:(j + 1) * C],
                                 sb_in["w_in2"][:], start=True, stop=True)
            o2 = work.tile([P, 4 * C], dt, tag="h3")
            gn_batch(p2[:, :T * C].rearrange("p (t c) -> p t c", c=C), T,
                     sb_in["g_in2"], sb_in["b_in2"], o2[:, :T * C],
                     relu=False)
            p3 = ps.tile([P, 4 * C], dt, tag="mmn")
            for j in range(T):
                nc.tensor.matmul(p3[:, j * C:(j + 1) * C],
                                 fs_t[:, j * P:(j + 1) * P],
                                 sb_in["w_int"][:], start=True, stop=True)
            o3 = work.tile([P, 4 * C], dt, tag="o3")
            gn_batch(p3[:, :T * C].rearrange("p (t c) -> p t c", c=C), T,
                     sb_in["g_int"], sb_in["b_int"], o3[:, :T * C],
                     relu=False)
            s = work.tile([P, 4 * C], dt, tag="s2")
            nc.vector.tensor_add(s[:, :T * C], o2[:, :T * C], o3[:, :T * C])
            nc.scalar.activation(feat[:, t0 * P:(t0 + T) * P], s[:, :T * C],
                                 AFT.Relu)

        def wnames_of(k):
            return ([f"fc1w{k}", f"fc2wa{k}", f"fc2wb{k}", f"linw{k}"],
                    [f"g_fc1{k}", f"b_fc1{k}", f"g_fc2{k}", f"b_fc2{k}",
                     f"g_lin{k}", f"b_lin{k}"])

        def load_idx(i):
            idx_t = idxp_pool.tile([P, max_cols], mybir.dt.int32, tag="idx")
            if cols16[i] > 0:
                nc.sync.dma_start(out=idx_t[:, :cols16[i]], in_=idx_p[i][:])
            return idx_t

        # tile groups of up to 4; TLH must be a group boundary
        groups = []
        t0 = 0
        while t0 < TL:
            T = min(4, TL - t0, (TLH - t0) if t0 < TLH else TL - t0)
            groups.append((t0, T))
            t0 += T
        # Pool-stream position at which AllGather-a(k+1) is emitted: late
        # enough (~75% of the block) that the first-half fc1 tiles it ships
        # are already computed, so the Pool engine does not stall there.
        agath = min((t0 + T for (t0, T) in groups
                     if t0 + T >= (TLH + TL) // 2), default=TLH)

        # ---------------- bootstrap: input block + block 0 fc1 -----------
        sb_in = load_weights(["w_in1", "w_int", "w_in2"],
                             ["g_in1", "b_in1", "g_in2", "b_in2",
                              "g_int", "b_int"])
        sbs = {0: load_weights(*wnames_of(0))}
        idxs = {0: load_idx(0)}
        for (t0, T) in groups:
            input_group(sb_in, t0, T)
            fc1_group(0, sbs[0], t0, T)
            if t0 + T == TLH:
                emit_allgather_a(0)
        emit_allgather_b(0)

        # ---------------- pipelined aggregation blocks -------------------
        for k in range(nblocks):
            i = k % n_scales
            if k + 1 < nblocks:
                sbs[k + 1] = load_weights(*wnames_of(k + 1))
                idxs[k + 1] = load_idx((k + 1) % n_scales)
            for (t0, T) in groups:
                for t in range(t0, t0 + T):
                    gather_tile(k, i, idxs[k], t)
                fc2_group(k, sbs[k], t0, T)
                if k + 1 < nblocks:
                    fc1_group(k + 1, sbs[k + 1], t0, T)
                    if t0 + T == agath:
                        emit_allgather_a(k + 1)
            if k + 1 < nblocks:
                emit_allgather_b(k + 1)
            del sbs[k], idxs[k]

        nc.sync.dma_start(out=out_p[:], in_=feat[:])

    _legalize_waits(nc)
    return nc


# ---------------------------------------------------------------------------
# input maps / output assembly
# ---------------------------------------------------------------------------

def _make_in_maps(inputs, meta, n_feat, n_blk, n_scales):
    feats = np.asarray(inputs["feats"], np.float32)
    NPL = meta["NPL"]
    order = meta["order"]
    nblocks = n_blk * n_scales

    shared = {
        "w_in1": np.asarray(inputs["in_w1"], np.float32),
        "w_int": np.asarray(inputs["in_wt"], np.float32),
        "w_in2": np.asarray(inputs["in_w2"], np.float32),
        "g_in1": _bc(inputs["in_g1"]), "b_in1": _bc(inputs["in_b1"]),
        "g_in2": _bc(inputs["in_g2"]), "b_in2": _bc(inputs["in_b2"]),
        "g_int": _bc(inputs["in_gt"]), "b_int": _bc(inputs["in_bt"]),
    }
    fc2w = np.asarray(inputs["fc2_w"], np.float32)
    for k in range(nblocks):
        shared[f"fc1w{k}"] = np.ascontiguousarray(
            np.asarray(inputs["fc1_w"], np.float32)[k])
        shared[f"fc2wa{k}"] = np.ascontiguousarray(fc2w[k, :C])
        shared[f"fc2wb{k}"] = np.ascontiguousarray(fc2w[k, C:])
        shared[f"linw{k}"] = np.ascontiguousarray(
            np.asarray(inputs["lin_w"], np.float32)[k])
        shared[f"g_fc1{k}"] = _bc(inputs["fc1_g"][k])
        shared[f"b_fc1{k}"] = _bc(inputs["fc1_b"][k])
        shared[f"g_fc2{k}"] = _bc(inputs["fc2_g"][k])
        shared[f"b_fc2{k}"] = _bc(inputs["fc2_b"][k])
        shared[f"g_lin{k}"] = _bc(inputs["lin_g"][k])
        shared[f"b_lin{k}"] = _bc(inputs["lin_b"][k])

    in_maps = []
    for c in range(NCORES):
        m = dict(shared)
        ft = np.zeros((n_feat, NPL), np.float32)
        valid = order[c] >= 0
        ft[:, valid] = feats[order[c][valid]].T
        m["featsT"] = np.ascontiguousarray(ft)
        for i in range(n_scales):
            m[f"idx{i}"] = meta["idx"][i][c]
        in_maps.append(m)
    return in_maps


def _assemble(outs, meta, n_nodes):
    TL, NPL = meta["TL"], meta["NPL"]
    order = meta["order"]
    full = np.zeros((n_nodes, C), np.float32)
    for c in range(NCORES):
        o = np.asarray(outs[c]["out"])           # [P, NPL]
        rows = o.reshape(P, TL, C).transpose(1, 0, 2).reshape(NPL, C)
        valid = order[c] >= 0
        full[order[c][valid]] = rows[valid]
    return full


# ---------------------------------------------------------------------------
# entry points
# ---------------------------------------------------------------------------

def forward(inputs, geom=None, runner="hw", trace=False):
    """Run the kernel. runner: 'hw' (Trainium via SPMD) or 'sim' (CoreSim)."""
    g = dict(FULL_GEOM)
    if geom:
        g.update(geom)
    n_nodes, n_feat = g["n_nodes"], g["n_feat"]
    n_blk, n_scales = g["n_blk"], g["n_scales"]

    meta = _host_prep(inputs["u"], inputs["v"], n_nodes)
    nc = _build(meta, n_feat, n_blk, n_scales)
    in_maps = _make_in_maps(inputs, meta, n_feat, n_blk, n_scales)

    info = {}
    if runner == "sim":
        from concourse.bass_interp import MultiCoreSim
        sim = MultiCoreSim(nc, NCORES)
        for c in range(NCORES):
            for k_, v_ in in_maps[c].items():
                sim.cores[c].tensor(k_)[:] = v_
        sim.simulate()
        outs = [{"out": sim.cores[c].tensor("out").copy()}
                for c in range(NCORES)]
    else:
        from concourse.bass_utils import run_bass_kernel_spmd
        res = run_bass_kernel_spmd(nc, in_maps, list(range(NCORES)),
                                   trace=trace)
        outs = res.results
        info["exec_time_ns"] = res.exec_time_ns
        info["profile_json"] = res.profile_json

    return _assemble(outs, meta, n_nodes), info


def forward_timed(inputs, geom=None, iters=3):
    """Like forward(runner='hw') but keeps the jitted SPMD executable and
    times repeated executions."""
    import time as _time

    import jax
    from jax.sharding import Mesh, PartitionSpec
    from jax.experimental.shard_map import shard_map
    from concourse import bass2jax

    g = dict(FULL_GEOM)
    if geom:
        g.update(geom)
    n_nodes, n_feat = g["n_nodes"], g["n_feat"]
    n_blk, n_scales = g["n_blk"], g["n_scales"]

    meta = _host_prep(inputs["u"], inputs["v"], n_nodes)
    nc = _build(meta, n_feat, n_blk, n_scales)
    in_maps = _make_in_maps(inputs, meta, n_feat, n_blk, n_scales)

    bass2jax.install_neuronx_cc_hook()
    nc.finalize()

    partition_name = (nc.partition_id_tensor.name
                      if nc.partition_id_tensor else None)
    import concourse.mybir as mb
    in_names, out_names, out_avals, zero_outs = [], [], [], []
    for alloc in nc.m.functions[0].allocations:
        if not isinstance(alloc, mb.MemoryLocationSet):
            continue
        name = alloc.memorylocations[0].name
        if alloc.kind == "ExternalInput":
            if name != partition_name:
                in_names.append(name)
        elif alloc.kind == "ExternalOutput":
            shape = tuple(alloc.tensor_shape)
            dtype = mb.dt.np(alloc.dtype)
            out_names.append(name)
            out_avals.append(jax.core.ShapedArray(shape, dtype))
            zero_outs.append(np.zeros(shape, dtype))
    n_params = len(in_names)
    n_outs = len(out_avals)
    in_names = in_names + out_names
    if partition_name is not None:
        in_names.append(partition_name)
    donate = tuple(range(n_params, n_params + n_outs))

    def _body(*args):
        operands = list(args)
        if partition_name is not None:
            operands.append(bass2jax.partition_id_tensor())
        outs = bass2jax._bass_exec_p.bind(
            *operands, out_avals=tuple(out_avals), in_names=tuple(in_names),
            out_names=tuple(out_names), lowering_input_output_aliases=(),
            sim_require_finite=True, sim_require_nnan=True, nc=nc)
        return tuple(outs)

    devices = jax.devices()[:NCORES]
    mesh = Mesh(np.asarray(devices), ("core",))
    sharded = jax.jit(
        shard_map(_body, mesh=mesh,
                  in_specs=(PartitionSpec("core"),) * (n_params + n_outs),
                  out_specs=(PartitionSpec("core"),) * n_outs,
                  check_rep=False),
        donate_argnums=donate, keep_unused=True)

    from jax.sharding import NamedSharding
    shard = NamedSharding(mesh, PartitionSpec("core"))
    concat_in = [jax.device_put(
        np.concatenate([np.asarray(in_maps[c][nm]) for c in range(NCORES)],
                       axis=0), shard) for nm in in_names[:n_params]]
    staged_zeros = [[jax.device_put(
        np.zeros((NCORES * z.shape[0], *z.shape[1:]), z.dtype), shard)
        for z in zero_outs] for _ in range(iters)]
    jax.block_until_ready(concat_in)
    jax.block_until_ready(staged_zeros)
    times = []
    out_arrs = None
    for it in range(iters):
        t0 = _time.time()
        out_arrs = sharded(*concat_in, *staged_zeros[it])
        jax.block_until_ready(out_arrs)
        times.append(time := _time.time() - t0)
        print(f"  iter {it}: {time*1e3:.2f} ms wall")
    outs = [{nm: np.asarray(out_arrs[j]).reshape(NCORES,
                                                 *out_avals[j].shape)[c]
             for j, nm in enumerate(out_names)} for c in range(NCORES)]
    full = _assemble(outs, meta, n_nodes)
    return full, dict(times=times, best_wall_s=min(times[1:])
                      if len(times) > 1 else times[0])


def kernel(**inputs) -> np.ndarray:
    out, _ = forward(inputs)
    return out
